# revision 45
# baseline (speedup 1.0000x reference)
"""Tree-GRU (arity-8, depth-5) over embedded leaves on 8 TRN2 NeuronCores.

Sharding: data-parallel over subtrees. Each core takes 4096 contiguous leaves
and runs levels 5..2 of the tree locally (512 -> 64 -> 8 -> 1 parents). The
root (level 1, 8 children = the 8 cores' level-2 outputs) is a trivial
16-matvec GRU done on host after gathering the per-core [384] outputs.

Device layout is feature-transposed: tensors live as [128 part, 3 ktile, ...]
with feature f = 128*k + p, so the GRU matmuls contract the partition dim.
Weights are host-pre-transposed into lhsT tiles; matmul dtype bf16 with fp32
PSUM accumulation; gates/elementwise fp32.

At the deepest level the leaf hidden state is zero, so step 0 skips the
recurrent matmuls, and the embedding gather (one indirect DMA per 128 leaves,
bf16 table) is overlapped with the PE transposes and the first GRU step. At
the small levels (<=64 parents, latency-bound) the gate biases are injected
into PSUM via a single K=3 matmul against a one-hot pattern (the only
start=True write, covering the whole PSUM bank), so the activation/vector
gate chain collapses to j-spanning instructions. The per-step output
accumulator is kept in child-major (permuted) order, and the 1/8 output-mean
scale is folded into a second, pre-scaled copy of W_ih used by the non-leaf
levels -- so the final step of each level writes the next level's input
directly as a fused raw-sum add, and the child-mean reduce for the next
initial hidden state is split per chunk to unblock the next level early.
"""

import numpy as np
import ml_dtypes

ARITY = 8
DIM = 384
VOCAB = 32000
NCORES = 8
P = 128
J = 3  # DIM // 128 feature tiles
N_LEAVES = 32768
LEAVES_CORE = N_LEAVES // NCORES  # 4096

BF16 = ml_dtypes.bfloat16

_PROG_CACHE = {}


def _levels_for(n_leaves_core):
    levels = []
    p = n_leaves_core // ARITY
    while p >= 1:
        levels.append(p)
        p //= ARITY
    assert levels[-1] == 1
    return levels


def _emit(tc, nc, aps, n_leaves_core):
    import concourse.mybir as mybir
    import concourse.bass as bass
    from concourse.masks import make_identity

    f32 = mybir.dt.float32
    bf16 = mybir.dt.bfloat16
    Sig = mybir.ActivationFunctionType.Sigmoid
    Tanh = mybir.ActivationFunctionType.Tanh
    Add = mybir.AluOpType.add
    Sub = mybir.AluOpType.subtract
    Mult = mybir.AluOpType.mult

    tokens, embed, wih_t, wih_s, whh_t, biases, biases_mm, onehot3, out_x, out_h = aps
    levels = _levels_for(n_leaves_core)
    P5 = levels[0]
    n_gtiles = n_leaves_core // P

    from contextlib import ExitStack

    with ExitStack() as ctx:
        const = ctx.enter_context(tc.tile_pool(name="const", bufs=1))
        xpool = ctx.enter_context(tc.tile_pool(name="xpool", bufs=1))
        state = ctx.enter_context(tc.tile_pool(name="state", bufs=1))
        gates = ctx.enter_context(tc.tile_pool(name="gates", bufs=3))
        gpool = ctx.enter_context(tc.tile_pool(name="gpool", bufs=1))
        pspool = ctx.enter_context(tc.tile_pool(name="pspool", bufs=8, space="PSUM"))

        # ---- tokens first, then kick off all gathers (DMA-bound prologue) ----
        tok_sb = const.tile([P, n_gtiles], mybir.dt.int32)
        # load tokens on the same dynamic queue as the gathers: the gathers'
        # dependency becomes pure queue order, no cross-ring semaphore wait
        nc.gpsimd.dma_start(tok_sb[:], tokens.rearrange("(g p) -> p g", p=P))
        ident = const.tile([P, P], bf16)
        make_identity(nc, ident[:])

        xgs = []
        for g in range(n_gtiles):
            xg = gpool.tile([P, DIM], bf16, name="xg", tag="xg", bufs=n_gtiles)
            gi_inst = nc.gpsimd.indirect_dma_start(
                out=xg[:],
                out_offset=None,
                in_=embed[:],
                in_offset=bass.IndirectOffsetOnAxis(ap=tok_sb[:, g : g + 1], axis=0),
            )
            if g % 2 == 1:
                gi_inst.ins.queue = "qPoolDynamic1"
            xgs.append(xg)

        # ---- constants / weights ----
        wih_sb = const.tile([P, J, 9, P], bf16)
        wih_s_sb = const.tile([P, J, 9, P], bf16)
        whh_sb = const.tile([P, J, 9, P], bf16)
        bias_sb = const.tile([P, 12], f32)
        bias3_sb = const.tile([3, 4, P], bf16)
        onehot3_sb = const.tile([3, 3, 512], bf16)
        nc.sync.dma_start(wih_sb[:], wih_t[:])
        nc.sync.dma_start(wih_s_sb[:], wih_s[:])
        nc.sync.dma_start(whh_sb[:], whh_t[:])
        nc.sync.dma_start(bias_sb[:], biases[:])
        nc.sync.dma_start(bias3_sb[:], biases_mm[:])
        nc.sync.dma_start(onehot3_sb[:], onehot3[:])

        x_in = {}
        for Pl in levels:
            x_in[Pl] = xpool.tile([P, J, ARITY, Pl], bf16, name=f"x{Pl}", tag=f"x{Pl}")

        # ---- transpose of gathered leaves happens inside the leaf level loop ----
        x5 = x_in[P5]

        def emit_transposes(g0, g1):
            for g in range(g0, g1):
                for j in range(J):
                    tp = pspool.tile([P, 512], bf16, name="tp", tag="ps")
                    nc.tensor.transpose(
                        tp[:, :P], xgs[g][:, j * P : (j + 1) * P], ident[:]
                    )
                    nc.vector.tensor_copy(
                        out=x5[:, j, :, 16 * g : 16 * (g + 1)],
                        in_=tp[:, :P].rearrange("p (par c) -> p c par", c=ARITY),
                    )

        def new_state(name, dtype, Pl):
            return state.tile([P, J, Pl], dtype, name=name, tag=name)

        def new_hacc(Pl):
            # permuted accumulator [128, J, 8, Pl/8] (child-major) when Pl >= 8
            if Pl >= ARITY:
                return state.tile([P, J, ARITY, Pl // ARITY], f32, name="hacc", tag="hacc")
            return state.tile([P, J, Pl], f32, name="hacc", tag="hacc")

        h = new_state("h", bf16, P5)
        hacc = new_hacc(P5)
        nc.gpsimd.memset(h[:], 0.0)
        nc.gpsimd.memset(hacc[:], 0.0)

        def psum_tile():
            return pspool.tile([P, 512], f32, name="ps", tag="ps")

        level_csum = [None]

        for li, Pl in enumerate(levels):
            with nc.named_scope(f"level_{Pl}"):
                xl = x_in[Pl]
                is_leaf = li == 0
                bias_in_psum = not is_leaf
                NCH = min(Pl, 256)
                nch = Pl // NCH
                g_per_ch = n_gtiles // nch
                for t in range(ARITY):
                    c = ARITY - 1 - t
                    skip_hh = is_leaf and t == 0
                    for ch in range(nch):
                        if is_leaf and t == 0:
                            emit_transposes(ch * g_per_ch, (ch + 1) * g_per_ch)
                        sl = slice(ch * NCH, (ch + 1) * NCH)
                        N3 = 3 * NCH

                        if bias_in_psum:
                            # one [128, 3*NCH] PSUM tile per role; the bias is a
                            # single K=3 matmul against a one-hot pattern that
                            # writes the whole tile (the only start=True), so
                            # gi/hh matmuls accumulate regardless of order.
                            ps_r, ps_z, ps_in = psum_tile(), psum_tile(), psum_tile()
                            ps_hn = None if skip_hh else psum_tile()

                            def view3(pst):
                                return pst[:, :N3].rearrange("p (j n) -> p j n", j=3)

                            def msl(pst, m):
                                return pst[:, m * NCH : (m + 1) * NCH]

                            role_list = [(ps_r, 0), (ps_z, 1), (ps_hn, 2), (ps_in, 3)]
                            for pst, ro in role_list:
                                if pst is None:
                                    continue
                                nc.tensor.matmul(
                                    pst[:, :N3],
                                    bias3_sb[:, ro, :],
                                    onehot3_sb[:, :, :NCH],
                                    start=True,
                                    stop=False,
                                )
                            # gi matmuls (no dependency on h)
                            for pst, moff in ((ps_r, 0), (ps_z, 3), (ps_in, 6)):
                                for m in range(3):
                                    for k in range(J):
                                        nc.tensor.matmul(
                                            msl(pst, m),
                                            wih_s_sb[:, k, moff + m, :],
                                            xl[:, k, c, sl],
                                            start=False,
                                            stop=(moff == 6 and m == 2 and k == 2),
                                        )
                            # hh matmuls
                            if not skip_hh:
                                for pst, moff in ((ps_r, 0), (ps_z, 3), (ps_hn, 6)):
                                    for m in range(3):
                                        for k in range(J):
                                            nc.tensor.matmul(
                                                msl(pst, m),
                                                whh_sb[:, k, moff + m, :],
                                                h[:, k, sl],
                                                start=False,
                                                stop=(m == 2 and k == 2),
                                            )
                        else:
                            # leaf level: per-m PSUM tiles (one bank each)
                            ps_r = [psum_tile()[:, :NCH] for _ in range(3)]
                            ps_z = [psum_tile()[:, :NCH] for _ in range(3)]
                            ps_in = [psum_tile()[:, :NCH] for _ in range(3)]
                            ps_hn = (
                                None
                                if skip_hh
                                else [psum_tile()[:, :NCH] for _ in range(3)]
                            )
                            for ps, moff in ((ps_r, 0), (ps_z, 3), (ps_in, 6)):
                                for m in range(3):
                                    for k in range(J):
                                        nc.tensor.matmul(
                                            ps[m],
                                            wih_sb[:, k, moff + m, :],
                                            xl[:, k, c, sl],
                                            start=(k == 0),
                                            stop=(k == 2 and (moff == 6 or skip_hh)),
                                        )
                            if not skip_hh:
                                for ps, moff in ((ps_r, 0), (ps_z, 3), (ps_hn, 6)):
                                    for m in range(3):
                                        for k in range(J):
                                            nc.tensor.matmul(
                                                ps[m],
                                                whh_sb[:, k, moff + m, :],
                                                h[:, k, sl],
                                                start=(k == 0 and moff == 6),
                                                stop=(k == 2),
                                            )

                        r_sb = gates.tile([P, J, NCH], bf16, name="r_sb", tag="r_sb")
                        z_sb = gates.tile([P, J, NCH], bf16, name="z_sb", tag="z_sb")
                        n_sb = gates.tile([P, J, NCH], bf16, name="n_sb", tag="n_sb")
                        rhn = gates.tile([P, J, NCH], f32, name="rhn", tag="rhn")
                        t1 = gates.tile([P, J, NCH], bf16, name="t1", tag="t1")

                        if bias_in_psum:
                            nc.scalar.activation(r_sb[:], view3(ps_r), Sig)
                            nc.scalar.activation(z_sb[:], view3(ps_z), Sig)
                            nc.vector.tensor_tensor(
                                out=rhn[:], in0=view3(ps_hn), in1=r_sb[:], op=Mult
                            )
                            nc.vector.tensor_tensor(
                                out=rhn[:], in0=rhn[:], in1=view3(ps_in), op=Add
                            )
                            nc.scalar.activation(n_sb[:], rhn[:], Tanh)
                        else:
                            for m in range(3):
                                nc.scalar.activation(
                                    r_sb[:, m], ps_r[m], Sig, bias=bias_sb[:, m : m + 1]
                                )
                            for m in range(3):
                                nc.scalar.activation(
                                    z_sb[:, m], ps_z[m], Sig, bias=bias_sb[:, 3 + m : 4 + m]
                                )
                            if skip_hh:
                                for m in range(3):
                                    nc.vector.tensor_scalar_mul(
                                        rhn[:, m], r_sb[:, m], bias_sb[:, 6 + m : 7 + m]
                                    )
                            else:
                                for m in range(3):
                                    nc.vector.scalar_tensor_tensor(
                                        out=rhn[:, m],
                                        in0=ps_hn[m],
                                        scalar=bias_sb[:, 6 + m : 7 + m],
                                        in1=r_sb[:, m],
                                        op0=Add,
                                        op1=Mult,
                                    )
                            for m in range(3):
                                nc.vector.tensor_tensor(
                                    out=rhn[:, m], in0=rhn[:, m], in1=ps_in[m], op=Add
                                )
                            for m in range(3):
                                nc.scalar.activation(
                                    n_sb[:, m], rhn[:, m], Tanh, bias=bias_sb[:, 9 + m : 10 + m]
                                )

                        # h = n + z * (h - n)
                        hsl = h[:, :, sl]
                        nc.vector.tensor_tensor(out=t1[:], in0=hsl, in1=n_sb[:], op=Sub)
                        nc.vector.tensor_tensor(out=t1[:], in0=z_sb[:], in1=t1[:], op=Mult)
                        nc.vector.tensor_tensor(out=hsl, in0=n_sb[:], in1=t1[:], op=Add)
                        if t == ARITY - 1 and Pl > 1:
                            hperm = hsl.rearrange("p j (q c) -> p j c q", c=ARITY)
                            qsl = slice(ch * NCH // ARITY, (ch + 1) * NCH // ARITY)
                            # child-mean of final hiddens for this chunk (h0 of
                            # the next level), before the x_next adds so the
                            # next level's recurrent matmuls unblock earlier
                            if ch == 0:
                                csum = state.tile(
                                    [P, J, Pl // ARITY], f32, name="csum", tag="csum"
                                )
                                level_csum[0] = csum
                            nc.vector.tensor_reduce(
                                out=level_csum[0][:, :, qsl],
                                in_=hsl.rearrange("p j (q c) -> p j q c", c=ARITY),
                                axis=mybir.AxisListType.X,
                                op=Add,
                            )
                            # final step: x_next = hacc + h (raw sum; the /8 is
                            # folded into wih_s), written straight into x_in
                            xn = x_in[Pl // ARITY]
                            for j in range(J):
                                eng = nc.gpsimd if j == 2 else nc.vector
                                eng.tensor_tensor(
                                    out=xn[:, j, :, qsl],
                                    in0=hacc[:, j, :, qsl],
                                    in1=hperm[:, j],
                                    op=Add,
                                )
                        elif Pl >= ARITY:
                            qsl = slice(ch * NCH // ARITY, (ch + 1) * NCH // ARITY)
                            nc.gpsimd.tensor_tensor(
                                out=hacc[:, :, :, qsl],
                                in0=hacc[:, :, :, qsl],
                                in1=hsl.rearrange("p j (q c) -> p j c q", c=ARITY),
                                op=Add,
                            )
                        else:
                            nc.gpsimd.tensor_tensor(
                                out=hacc[:, :, sl], in0=hacc[:, :, sl], in1=hsl, op=Add
                            )

                # ---- level epilogue ----
                if Pl > 1:
                    Pn = Pl // ARITY
                    csum = level_csum[0]
                    h = new_state("h", bf16, Pn)
                    hacc = new_hacc(Pn)
                    nc.scalar.mul(h[:], csum[:], 1.0 / ARITY)
                    nc.gpsimd.memset(hacc[:], 0.0)
                else:
                    xo = state.tile([P, J], f32, name="xo", tag="xo")
                    nc.scalar.mul(xo[:], hacc[:, :, 0], 1.0 / ARITY)
                    ho = state.tile([P, J], f32, name="ho", tag="ho")
                    nc.vector.tensor_copy(ho[:], h[:, :, 0])
                    nc.sync.dma_start(out_x[:], xo[:])
                    nc.sync.dma_start(out_h[:], ho[:])


def _build_program(n_leaves_core):
    if n_leaves_core in _PROG_CACHE:
        return _PROG_CACHE[n_leaves_core]
    import concourse.bacc as bacc
    import concourse.mybir as mybir
    import concourse.tile as tile

    f32 = mybir.dt.float32
    bf16 = mybir.dt.bfloat16

    nc = bacc.Bacc(
        "TRN2",
        target_bir_lowering=False,
        debug=False,
        enable_asserts=False,
        num_devices=NCORES,
        num_swdge_queues=2,
    )
    tokens = nc.dram_tensor("tokens", [n_leaves_core], mybir.dt.int32, kind="ExternalInput").ap()
    embed = nc.dram_tensor("embed", [VOCAB, DIM], bf16, kind="ExternalInput").ap()
    wih_t = nc.dram_tensor("wih_t", [P, J, 9, P], bf16, kind="ExternalInput").ap()
    wih_s = nc.dram_tensor("wih_s", [P, J, 9, P], bf16, kind="ExternalInput").ap()
    whh_t = nc.dram_tensor("whh_t", [P, J, 9, P], bf16, kind="ExternalInput").ap()
    biases = nc.dram_tensor("biases", [P, 12], f32, kind="ExternalInput").ap()
    biases_mm = nc.dram_tensor("biases_mm", [3, 4, P], bf16, kind="ExternalInput").ap()
    onehot3 = nc.dram_tensor("onehot3", [3, 3, 512], bf16, kind="ExternalInput").ap()
    out_x = nc.dram_tensor("out_x", [P, J], f32, kind="ExternalOutput").ap()
    out_h = nc.dram_tensor("out_h", [P, J], f32, kind="ExternalOutput").ap()

    with tile.TileContext(nc) as tc:
        _emit(
            tc,
            nc,
            (tokens, embed, wih_t, wih_s, whh_t, biases, biases_mm, onehot3, out_x, out_h),
            n_leaves_core,
        )
    nc.compile()
    _PROG_CACHE[n_leaves_core] = nc
    return nc


def _retile_weights(w):
    # w: [1152, 384] -> lhsT tiles [128(k_part), 3(k), 9(m), 128(m_col)] bf16
    wt = np.ascontiguousarray(w.T)  # [384, 1152]
    wt = wt.reshape(J, P, 9, P).transpose(1, 0, 2, 3)
    return np.ascontiguousarray(wt).astype(BF16)


def _prep_bias(b_ih, b_hh):
    biases = np.zeros((P, 12), np.float32)
    comb = (b_ih + b_hh).reshape(9, P)
    biases[:, 0:6] = comb[0:6].T
    biases[:, 6:9] = b_hh.reshape(9, P)[6:9].T
    biases[:, 9:12] = b_ih.reshape(9, P)[6:9].T
    return biases


def _prep_bias_mm(b_ih, b_hh):
    # lhsT[k, ro, q] = bias[q, 3*ro + k]: the K=3 bias matmul against the
    # one-hot rhs yields out[q, (j, n)] = bias[q, 3*ro + j].
    b = _prep_bias(b_ih, b_hh)  # [128, 12] cols: r0..2 z0..2 hn0..2 in0..2
    out = b.T.reshape(4, 3, P).transpose(1, 0, 2)
    return np.ascontiguousarray(out).astype(BF16)


def _prep_onehot3():
    out = np.zeros((3, 3, 512), np.float32)
    for k in range(3):
        out[k, k, :] = 1.0
    return out.astype(BF16)


def _gru_gates(x_t, h, w_ih, w_hh, b_ih, b_hh):
    gi = x_t @ w_ih.T + b_ih
    gh = h @ w_hh.T + b_hh
    i_r, i_z, i_n = np.split(gi, 3, axis=-1)
    h_r, h_z, h_n = np.split(gh, 3, axis=-1)
    r = 1.0 / (1.0 + np.exp(-(i_r + h_r)))
    z = 1.0 / (1.0 + np.exp(-(i_z + h_z)))
    n = np.tanh(i_n + r * h_n)
    return (1.0 - z) * n + z * h


def _root_gru(x_children, h0, w_ih, w_hh, b_ih, b_hh):
    h = h0.astype(np.float64)
    acc = np.zeros_like(h)
    for t in range(ARITY):
        x_t = x_children[ARITY - 1 - t].astype(np.float64)
        h = _gru_gates(x_t, h, w_ih.astype(np.float64), w_hh.astype(np.float64),
                       b_ih.astype(np.float64), b_hh.astype(np.float64))
        acc += h
    return (acc / ARITY).astype(np.float32)


def kernel(leaf_tokens, embed_table, w_ih, w_hh, b_ih, b_hh):
    from concourse.bass_utils import run_bass_kernel_spmd

    leaf_tokens = np.asarray(leaf_tokens, np.int32)
    embed_table = np.asarray(embed_table, np.float32)
    w_ih = np.asarray(w_ih, np.float32)
    w_hh = np.asarray(w_hh, np.float32)
    b_ih = np.asarray(b_ih, np.float32)
    b_hh = np.asarray(b_hh, np.float32)

    nc = _build_program(LEAVES_CORE)

    embed_bf = embed_table.astype(BF16)
    wih_t = _retile_weights(w_ih)
    wih_s = _retile_weights(w_ih / ARITY)
    whh_t = _retile_weights(w_hh)
    biases = _prep_bias(b_ih, b_hh)
    biases_mm = _prep_bias_mm(b_ih, b_hh)
    in_maps = []
    for core in range(NCORES):
        in_maps.append(
            {
                "tokens": np.ascontiguousarray(
                    leaf_tokens[core * LEAVES_CORE : (core + 1) * LEAVES_CORE]
                ),
                "embed": embed_bf,
                "wih_t": wih_t,
                "wih_s": wih_s,
                "whh_t": whh_t,
                "biases": biases,
                "biases_mm": biases_mm,
                "onehot3": _prep_onehot3(),
            }
        )
    res = run_bass_kernel_spmd(nc, in_maps, core_ids=list(range(NCORES)))

    xs = np.zeros((NCORES, DIM), np.float32)
    h8 = np.zeros((NCORES, DIM), np.float32)
    for core in range(NCORES):
        xs[core] = res.results[core]["out_x"].T.reshape(-1)
        h8[core] = res.results[core]["out_h"].T.reshape(-1)

    h0 = h8.mean(axis=0)
    out = _root_gru(xs, h0, w_ih, w_hh, b_ih, b_hh)
    return out.reshape(1, 1, DIM)


# revision 46
# speedup vs baseline: 1.0161x; 1.0161x over previous
"""Tree-GRU (arity-8, depth-5) over embedded leaves on 8 TRN2 NeuronCores.

Sharding: data-parallel over subtrees. Each core takes 4096 contiguous leaves
and runs levels 5..2 of the tree locally (512 -> 64 -> 8 -> 1 parents). The
root (level 1, 8 children = the 8 cores' level-2 outputs) is a trivial
16-matvec GRU done on host after gathering the per-core [384] outputs.

Device layout is feature-transposed: tensors live as [128 part, 3 ktile, ...]
with feature f = 128*k + p, so the GRU matmuls contract the partition dim.
Weights are host-pre-transposed into lhsT tiles; matmul dtype bf16 with fp32
PSUM accumulation; gates/elementwise fp32.

At the deepest level the leaf hidden state is zero, so step 0 skips the
recurrent matmuls, and the embedding gather (one indirect DMA per 128 leaves,
bf16 table) is overlapped with the PE transposes and the first GRU step. At
the small levels (<=64 parents, latency-bound) the gate biases are injected
into PSUM via a single K=3 matmul against a one-hot pattern (the only
start=True write, covering the whole PSUM bank), so the activation/vector
gate chain collapses to j-spanning instructions. The per-step output
accumulator is kept in child-major (permuted) order, and the 1/8 output-mean
scale is folded into a second, pre-scaled copy of W_ih used by the non-leaf
levels -- so the final step of each level writes the next level's input
directly as a fused raw-sum add, and the child-mean reduce for the next
initial hidden state is split per chunk to unblock the next level early.
"""

import numpy as np
import ml_dtypes

ARITY = 8
DIM = 384
VOCAB = 32000
NCORES = 8
P = 128
J = 3  # DIM // 128 feature tiles
N_LEAVES = 32768
LEAVES_CORE = N_LEAVES // NCORES  # 4096

BF16 = ml_dtypes.bfloat16

_PROG_CACHE = {}


def _levels_for(n_leaves_core):
    levels = []
    p = n_leaves_core // ARITY
    while p >= 1:
        levels.append(p)
        p //= ARITY
    assert levels[-1] == 1
    return levels


def _emit(tc, nc, aps, n_leaves_core):
    import concourse.mybir as mybir
    import concourse.bass as bass
    from concourse.masks import make_identity

    f32 = mybir.dt.float32
    bf16 = mybir.dt.bfloat16
    Sig = mybir.ActivationFunctionType.Sigmoid
    Tanh = mybir.ActivationFunctionType.Tanh
    Add = mybir.AluOpType.add
    Sub = mybir.AluOpType.subtract
    Mult = mybir.AluOpType.mult

    tokens, embed, wih_t, wih_s, whh_t, biases, biases_mm, onehot3, out_x, out_h = aps
    levels = _levels_for(n_leaves_core)
    P5 = levels[0]
    n_gtiles = n_leaves_core // P

    from contextlib import ExitStack

    with ExitStack() as ctx:
        const = ctx.enter_context(tc.tile_pool(name="const", bufs=1))
        xpool = ctx.enter_context(tc.tile_pool(name="xpool", bufs=1))
        state = ctx.enter_context(tc.tile_pool(name="state", bufs=1))
        gates = ctx.enter_context(tc.tile_pool(name="gates", bufs=3))
        gpool = ctx.enter_context(tc.tile_pool(name="gpool", bufs=1))
        pspool = ctx.enter_context(tc.tile_pool(name="pspool", bufs=8, space="PSUM"))

        # ---- tokens first, then kick off all gathers (DMA-bound prologue) ----
        tok_sb = const.tile([P, n_gtiles], mybir.dt.int32)
        nc.sync.dma_start(tok_sb[:], tokens.rearrange("(g p) -> p g", p=P))
        ident = const.tile([P, P], bf16)
        make_identity(nc, ident[:])

        xgs = []
        for g in range(n_gtiles):
            xg = gpool.tile([P, DIM], bf16, name="xg", tag="xg", bufs=n_gtiles)
            gi_inst = nc.gpsimd.indirect_dma_start(
                out=xg[:],
                out_offset=None,
                in_=embed[:],
                in_offset=bass.IndirectOffsetOnAxis(ap=tok_sb[:, g : g + 1], axis=0),
            )
            if g % 2 == 1:
                gi_inst.ins.queue = "qPoolDynamic1"
            xgs.append(xg)

        # ---- constants / weights ----
        wih_sb = const.tile([P, J, 9, P], bf16)
        wih_s_sb = const.tile([P, J, 9, P], bf16)
        whh_sb = const.tile([P, J, 9, P], bf16)
        bias_sb = const.tile([P, 12], f32)
        bias3_sb = const.tile([3, 4, P], bf16)
        onehot3_sb = const.tile([3, 3, 512], bf16)
        nc.sync.dma_start(wih_sb[:], wih_t[:])
        nc.sync.dma_start(wih_s_sb[:], wih_s[:])
        nc.sync.dma_start(whh_sb[:], whh_t[:])
        nc.sync.dma_start(bias_sb[:], biases[:])
        nc.sync.dma_start(bias3_sb[:], biases_mm[:])
        nc.sync.dma_start(onehot3_sb[:], onehot3[:])

        x_in = {}
        for Pl in levels:
            x_in[Pl] = xpool.tile([P, J, ARITY, Pl], bf16, name=f"x{Pl}", tag=f"x{Pl}")

        # ---- transpose of gathered leaves happens inside the leaf level loop ----
        x5 = x_in[P5]

        def emit_transposes(g0, g1):
            for g in range(g0, g1):
                for j in range(J):
                    tp = pspool.tile([P, 512], bf16, name="tp", tag="ps")
                    nc.tensor.transpose(
                        tp[:, :P], xgs[g][:, j * P : (j + 1) * P], ident[:]
                    )
                    nc.vector.tensor_copy(
                        out=x5[:, j, :, 16 * g : 16 * (g + 1)],
                        in_=tp[:, :P].rearrange("p (par c) -> p c par", c=ARITY),
                    )

        def new_state(name, dtype, Pl):
            return state.tile([P, J, Pl], dtype, name=name, tag=name)

        def new_hacc(Pl):
            # permuted accumulator [128, J, 8, Pl/8] (child-major) when Pl >= 8
            if Pl >= ARITY:
                return state.tile([P, J, ARITY, Pl // ARITY], f32, name="hacc", tag="hacc")
            return state.tile([P, J, Pl], f32, name="hacc", tag="hacc")

        h = new_state("h", bf16, P5)
        hacc = new_hacc(P5)
        nc.gpsimd.memset(h[:], 0.0)
        nc.gpsimd.memset(hacc[:], 0.0)

        def psum_tile():
            return pspool.tile([P, 512], f32, name="ps", tag="ps")

        level_csum = [None]

        for li, Pl in enumerate(levels):
            with nc.named_scope(f"level_{Pl}"):
                xl = x_in[Pl]
                is_leaf = li == 0
                bias_in_psum = not is_leaf
                NCH = min(Pl, 256)
                nch = Pl // NCH
                g_per_ch = n_gtiles // nch
                for t in range(ARITY):
                    c = ARITY - 1 - t
                    skip_hh = is_leaf and t == 0
                    for ch in range(nch):
                        if is_leaf and t == 0:
                            emit_transposes(ch * g_per_ch, (ch + 1) * g_per_ch)
                        sl = slice(ch * NCH, (ch + 1) * NCH)
                        N3 = 3 * NCH

                        if bias_in_psum:
                            # one [128, 3*NCH] PSUM tile per role; the bias is a
                            # single K=3 matmul against a one-hot pattern that
                            # writes the whole tile (the only start=True), so
                            # gi/hh matmuls accumulate regardless of order.
                            ps_r, ps_z, ps_in = psum_tile(), psum_tile(), psum_tile()
                            ps_hn = None if skip_hh else psum_tile()

                            def view3(pst):
                                return pst[:, :N3].rearrange("p (j n) -> p j n", j=3)

                            def msl(pst, m):
                                return pst[:, m * NCH : (m + 1) * NCH]

                            role_list = [(ps_r, 0), (ps_z, 1), (ps_hn, 2), (ps_in, 3)]
                            for pst, ro in role_list:
                                if pst is None:
                                    continue
                                nc.tensor.matmul(
                                    pst[:, :N3],
                                    bias3_sb[:, ro, :],
                                    onehot3_sb[:, :, :NCH],
                                    start=True,
                                    stop=False,
                                )
                            # gi matmuls (no dependency on h)
                            for pst, moff in ((ps_r, 0), (ps_z, 3), (ps_in, 6)):
                                for m in range(3):
                                    for k in range(J):
                                        nc.tensor.matmul(
                                            msl(pst, m),
                                            wih_s_sb[:, k, moff + m, :],
                                            xl[:, k, c, sl],
                                            start=False,
                                            stop=(moff == 6 and m == 2 and k == 2),
                                        )
                            # hh matmuls
                            if not skip_hh:
                                for pst, moff in ((ps_r, 0), (ps_z, 3), (ps_hn, 6)):
                                    for m in range(3):
                                        for k in range(J):
                                            nc.tensor.matmul(
                                                msl(pst, m),
                                                whh_sb[:, k, moff + m, :],
                                                h[:, k, sl],
                                                start=False,
                                                stop=(m == 2 and k == 2),
                                            )
                        else:
                            # leaf level: per-m PSUM tiles (one bank each)
                            ps_r = [psum_tile()[:, :NCH] for _ in range(3)]
                            ps_z = [psum_tile()[:, :NCH] for _ in range(3)]
                            ps_in = [psum_tile()[:, :NCH] for _ in range(3)]
                            ps_hn = (
                                None
                                if skip_hh
                                else [psum_tile()[:, :NCH] for _ in range(3)]
                            )
                            for ps, moff in ((ps_r, 0), (ps_z, 3), (ps_in, 6)):
                                for m in range(3):
                                    for k in range(J):
                                        nc.tensor.matmul(
                                            ps[m],
                                            wih_sb[:, k, moff + m, :],
                                            xl[:, k, c, sl],
                                            start=(k == 0),
                                            stop=(k == 2 and (moff == 6 or skip_hh)),
                                        )
                            if not skip_hh:
                                for ps, moff in ((ps_r, 0), (ps_z, 3), (ps_hn, 6)):
                                    for m in range(3):
                                        for k in range(J):
                                            nc.tensor.matmul(
                                                ps[m],
                                                whh_sb[:, k, moff + m, :],
                                                h[:, k, sl],
                                                start=(k == 0 and moff == 6),
                                                stop=(k == 2),
                                            )

                        r_sb = gates.tile([P, J, NCH], bf16, name="r_sb", tag="r_sb")
                        z_sb = gates.tile([P, J, NCH], bf16, name="z_sb", tag="z_sb")
                        n_sb = gates.tile([P, J, NCH], bf16, name="n_sb", tag="n_sb")
                        rhn = gates.tile([P, J, NCH], f32, name="rhn", tag="rhn")
                        t1 = gates.tile([P, J, NCH], bf16, name="t1", tag="t1")

                        if bias_in_psum:
                            nc.scalar.activation(r_sb[:], view3(ps_r), Sig)
                            nc.scalar.activation(z_sb[:], view3(ps_z), Sig)
                            nc.vector.tensor_tensor(
                                out=rhn[:], in0=view3(ps_hn), in1=r_sb[:], op=Mult
                            )
                            nc.vector.tensor_tensor(
                                out=rhn[:], in0=rhn[:], in1=view3(ps_in), op=Add
                            )
                            nc.scalar.activation(n_sb[:], rhn[:], Tanh)
                        else:
                            for m in range(3):
                                nc.scalar.activation(
                                    r_sb[:, m], ps_r[m], Sig, bias=bias_sb[:, m : m + 1]
                                )
                            for m in range(3):
                                nc.scalar.activation(
                                    z_sb[:, m], ps_z[m], Sig, bias=bias_sb[:, 3 + m : 4 + m]
                                )
                            if skip_hh:
                                for m in range(3):
                                    nc.vector.tensor_scalar_mul(
                                        rhn[:, m], r_sb[:, m], bias_sb[:, 6 + m : 7 + m]
                                    )
                            else:
                                for m in range(3):
                                    nc.vector.scalar_tensor_tensor(
                                        out=rhn[:, m],
                                        in0=ps_hn[m],
                                        scalar=bias_sb[:, 6 + m : 7 + m],
                                        in1=r_sb[:, m],
                                        op0=Add,
                                        op1=Mult,
                                    )
                            for m in range(3):
                                nc.vector.tensor_tensor(
                                    out=rhn[:, m], in0=rhn[:, m], in1=ps_in[m], op=Add
                                )
                            for m in range(3):
                                nc.scalar.activation(
                                    n_sb[:, m], rhn[:, m], Tanh, bias=bias_sb[:, 9 + m : 10 + m]
                                )

                        # h = n + z * (h - n)
                        hsl = h[:, :, sl]
                        nc.vector.tensor_tensor(out=t1[:], in0=hsl, in1=n_sb[:], op=Sub)
                        nc.vector.tensor_tensor(out=t1[:], in0=z_sb[:], in1=t1[:], op=Mult)
                        nc.vector.tensor_tensor(out=hsl, in0=n_sb[:], in1=t1[:], op=Add)
                        if t == ARITY - 1 and Pl > 1:
                            hperm = hsl.rearrange("p j (q c) -> p j c q", c=ARITY)
                            qsl = slice(ch * NCH // ARITY, (ch + 1) * NCH // ARITY)
                            # child-mean of final hiddens for this chunk (h0 of
                            # the next level), before the x_next adds so the
                            # next level's recurrent matmuls unblock earlier
                            if ch == 0:
                                csum = state.tile(
                                    [P, J, Pl // ARITY], f32, name="csum", tag="csum"
                                )
                                level_csum[0] = csum
                            nc.vector.tensor_reduce(
                                out=level_csum[0][:, :, qsl],
                                in_=hsl.rearrange("p j (q c) -> p j q c", c=ARITY),
                                axis=mybir.AxisListType.X,
                                op=Add,
                            )
                            # final step: x_next = hacc + h (raw sum; the /8 is
                            # folded into wih_s), written straight into x_in
                            xn = x_in[Pl // ARITY]
                            for j in range(J):
                                eng = nc.gpsimd if j == 2 else nc.vector
                                eng.tensor_tensor(
                                    out=xn[:, j, :, qsl],
                                    in0=hacc[:, j, :, qsl],
                                    in1=hperm[:, j],
                                    op=Add,
                                )
                        elif Pl >= ARITY:
                            qsl = slice(ch * NCH // ARITY, (ch + 1) * NCH // ARITY)
                            nc.gpsimd.tensor_tensor(
                                out=hacc[:, :, :, qsl],
                                in0=hacc[:, :, :, qsl],
                                in1=hsl.rearrange("p j (q c) -> p j c q", c=ARITY),
                                op=Add,
                            )
                        else:
                            nc.gpsimd.tensor_tensor(
                                out=hacc[:, :, sl], in0=hacc[:, :, sl], in1=hsl, op=Add
                            )

                # ---- level epilogue ----
                if Pl > 1:
                    Pn = Pl // ARITY
                    csum = level_csum[0]
                    h = new_state("h", bf16, Pn)
                    hacc = new_hacc(Pn)
                    nc.scalar.mul(h[:], csum[:], 1.0 / ARITY)
                    nc.gpsimd.memset(hacc[:], 0.0)
                else:
                    xo = state.tile([P, J], f32, name="xo", tag="xo")
                    nc.scalar.mul(xo[:], hacc[:, :, 0], 1.0 / ARITY)
                    ho = state.tile([P, J], f32, name="ho", tag="ho")
                    nc.vector.tensor_copy(ho[:], h[:, :, 0])
                    nc.sync.dma_start(out_x[:], xo[:])
                    nc.sync.dma_start(out_h[:], ho[:])


def _build_program(n_leaves_core):
    if n_leaves_core in _PROG_CACHE:
        return _PROG_CACHE[n_leaves_core]
    import concourse.bacc as bacc
    import concourse.mybir as mybir
    import concourse.tile as tile

    f32 = mybir.dt.float32
    bf16 = mybir.dt.bfloat16

    nc = bacc.Bacc(
        "TRN2",
        target_bir_lowering=False,
        debug=False,
        enable_asserts=False,
        num_devices=NCORES,
        num_swdge_queues=2,
    )
    tokens = nc.dram_tensor("tokens", [n_leaves_core], mybir.dt.int32, kind="ExternalInput").ap()
    embed = nc.dram_tensor("embed", [VOCAB, DIM], bf16, kind="ExternalInput").ap()
    wih_t = nc.dram_tensor("wih_t", [P, J, 9, P], bf16, kind="ExternalInput").ap()
    wih_s = nc.dram_tensor("wih_s", [P, J, 9, P], bf16, kind="ExternalInput").ap()
    whh_t = nc.dram_tensor("whh_t", [P, J, 9, P], bf16, kind="ExternalInput").ap()
    biases = nc.dram_tensor("biases", [P, 12], f32, kind="ExternalInput").ap()
    biases_mm = nc.dram_tensor("biases_mm", [3, 4, P], bf16, kind="ExternalInput").ap()
    onehot3 = nc.dram_tensor("onehot3", [3, 3, 512], bf16, kind="ExternalInput").ap()
    out_x = nc.dram_tensor("out_x", [P, J], f32, kind="ExternalOutput").ap()
    out_h = nc.dram_tensor("out_h", [P, J], f32, kind="ExternalOutput").ap()

    with tile.TileContext(nc) as tc:
        _emit(
            tc,
            nc,
            (tokens, embed, wih_t, wih_s, whh_t, biases, biases_mm, onehot3, out_x, out_h),
            n_leaves_core,
        )
    nc.compile()
    _PROG_CACHE[n_leaves_core] = nc
    return nc


def _retile_weights(w):
    # w: [1152, 384] -> lhsT tiles [128(k_part), 3(k), 9(m), 128(m_col)] bf16
    wt = np.ascontiguousarray(w.T)  # [384, 1152]
    wt = wt.reshape(J, P, 9, P).transpose(1, 0, 2, 3)
    return np.ascontiguousarray(wt).astype(BF16)


def _prep_bias(b_ih, b_hh):
    biases = np.zeros((P, 12), np.float32)
    comb = (b_ih + b_hh).reshape(9, P)
    biases[:, 0:6] = comb[0:6].T
    biases[:, 6:9] = b_hh.reshape(9, P)[6:9].T
    biases[:, 9:12] = b_ih.reshape(9, P)[6:9].T
    return biases


def _prep_bias_mm(b_ih, b_hh):
    # lhsT[k, ro, q] = bias[q, 3*ro + k]: the K=3 bias matmul against the
    # one-hot rhs yields out[q, (j, n)] = bias[q, 3*ro + j].
    b = _prep_bias(b_ih, b_hh)  # [128, 12] cols: r0..2 z0..2 hn0..2 in0..2
    out = b.T.reshape(4, 3, P).transpose(1, 0, 2)
    return np.ascontiguousarray(out).astype(BF16)


def _prep_onehot3():
    out = np.zeros((3, 3, 512), np.float32)
    for k in range(3):
        out[k, k, :] = 1.0
    return out.astype(BF16)


def _gru_gates(x_t, h, w_ih, w_hh, b_ih, b_hh):
    gi = x_t @ w_ih.T + b_ih
    gh = h @ w_hh.T + b_hh
    i_r, i_z, i_n = np.split(gi, 3, axis=-1)
    h_r, h_z, h_n = np.split(gh, 3, axis=-1)
    r = 1.0 / (1.0 + np.exp(-(i_r + h_r)))
    z = 1.0 / (1.0 + np.exp(-(i_z + h_z)))
    n = np.tanh(i_n + r * h_n)
    return (1.0 - z) * n + z * h


def _root_gru(x_children, h0, w_ih, w_hh, b_ih, b_hh):
    h = h0.astype(np.float64)
    acc = np.zeros_like(h)
    for t in range(ARITY):
        x_t = x_children[ARITY - 1 - t].astype(np.float64)
        h = _gru_gates(x_t, h, w_ih.astype(np.float64), w_hh.astype(np.float64),
                       b_ih.astype(np.float64), b_hh.astype(np.float64))
        acc += h
    return (acc / ARITY).astype(np.float32)


def kernel(leaf_tokens, embed_table, w_ih, w_hh, b_ih, b_hh):
    from concourse.bass_utils import run_bass_kernel_spmd

    leaf_tokens = np.asarray(leaf_tokens, np.int32)
    embed_table = np.asarray(embed_table, np.float32)
    w_ih = np.asarray(w_ih, np.float32)
    w_hh = np.asarray(w_hh, np.float32)
    b_ih = np.asarray(b_ih, np.float32)
    b_hh = np.asarray(b_hh, np.float32)

    nc = _build_program(LEAVES_CORE)

    embed_bf = embed_table.astype(BF16)
    wih_t = _retile_weights(w_ih)
    wih_s = _retile_weights(w_ih / ARITY)
    whh_t = _retile_weights(w_hh)
    biases = _prep_bias(b_ih, b_hh)
    biases_mm = _prep_bias_mm(b_ih, b_hh)
    in_maps = []
    for core in range(NCORES):
        in_maps.append(
            {
                "tokens": np.ascontiguousarray(
                    leaf_tokens[core * LEAVES_CORE : (core + 1) * LEAVES_CORE]
                ),
                "embed": embed_bf,
                "wih_t": wih_t,
                "wih_s": wih_s,
                "whh_t": whh_t,
                "biases": biases,
                "biases_mm": biases_mm,
                "onehot3": _prep_onehot3(),
            }
        )
    res = run_bass_kernel_spmd(nc, in_maps, core_ids=list(range(NCORES)))

    xs = np.zeros((NCORES, DIM), np.float32)
    h8 = np.zeros((NCORES, DIM), np.float32)
    for core in range(NCORES):
        xs[core] = res.results[core]["out_x"].T.reshape(-1)
        h8[core] = res.results[core]["out_h"].T.reshape(-1)

    h0 = h8.mean(axis=0)
    out = _root_gru(xs, h0, w_ih, w_hh, b_ih, b_hh)
    return out.reshape(1, 1, DIM)


# revision 47
# speedup vs baseline: 1.0224x; 1.0062x over previous
"""Tree-GRU (arity-8, depth-5) over embedded leaves on 8 TRN2 NeuronCores.

Sharding: data-parallel over subtrees. Each core takes 4096 contiguous leaves
and runs levels 5..2 of the tree locally (512 -> 64 -> 8 -> 1 parents). The
root (level 1, 8 children = the 8 cores' level-2 outputs) is a trivial
16-matvec GRU done on host after gathering the per-core [384] outputs.

Device layout is feature-transposed: tensors live as [128 part, 3 ktile, ...]
with feature f = 128*k + p, so the GRU matmuls contract the partition dim.
Weights are host-pre-transposed into lhsT tiles; matmul dtype bf16 with fp32
PSUM accumulation; gates/elementwise fp32.

At the deepest level the leaf hidden state is zero, so step 0 skips the
recurrent matmuls, and the embedding gather (one indirect DMA per 128 leaves,
bf16 table) is overlapped with the PE transposes and the first GRU step. At
the small levels (<=64 parents, latency-bound) the gate biases are injected
into PSUM via a single K=3 matmul against a one-hot pattern (the only
start=True write, covering the whole PSUM bank), so the activation/vector
gate chain collapses to j-spanning instructions. The per-step output
accumulator is kept in child-major (permuted) order, and the 1/8 output-mean
scale is folded into a second, pre-scaled copy of W_ih used by the non-leaf
levels -- so the final step of each level writes the next level's input
directly as a fused raw-sum add, and the child-mean reduce for the next
initial hidden state is split per chunk to unblock the next level early.
"""

import numpy as np
import ml_dtypes

ARITY = 8
DIM = 384
VOCAB = 32000
NCORES = 8
P = 128
J = 3  # DIM // 128 feature tiles
N_LEAVES = 32768
LEAVES_CORE = N_LEAVES // NCORES  # 4096

BF16 = ml_dtypes.bfloat16

_PROG_CACHE = {}


def _levels_for(n_leaves_core):
    levels = []
    p = n_leaves_core // ARITY
    while p >= 1:
        levels.append(p)
        p //= ARITY
    assert levels[-1] == 1
    return levels


def _emit(tc, nc, aps, n_leaves_core):
    import concourse.mybir as mybir
    import concourse.bass as bass
    from concourse.masks import make_identity

    f32 = mybir.dt.float32
    bf16 = mybir.dt.bfloat16
    Sig = mybir.ActivationFunctionType.Sigmoid
    Tanh = mybir.ActivationFunctionType.Tanh
    Add = mybir.AluOpType.add
    Sub = mybir.AluOpType.subtract
    Mult = mybir.AluOpType.mult

    tokens, embed, wih_t, wih_s, whh_t, biases, biases_mm, onehot3, out_x, out_h = aps
    levels = _levels_for(n_leaves_core)
    P5 = levels[0]
    n_gtiles = n_leaves_core // P

    from contextlib import ExitStack

    with ExitStack() as ctx:
        const = ctx.enter_context(tc.tile_pool(name="const", bufs=1))
        xpool = ctx.enter_context(tc.tile_pool(name="xpool", bufs=1))
        state = ctx.enter_context(tc.tile_pool(name="state", bufs=1))
        gates = ctx.enter_context(tc.tile_pool(name="gates", bufs=4))
        gpool = ctx.enter_context(tc.tile_pool(name="gpool", bufs=1))
        pspool = ctx.enter_context(tc.tile_pool(name="pspool", bufs=8, space="PSUM"))

        # ---- tokens first, then kick off all gathers (DMA-bound prologue) ----
        tok_sb = const.tile([P, n_gtiles], mybir.dt.int32)
        nc.sync.dma_start(tok_sb[:], tokens.rearrange("(g p) -> p g", p=P))
        ident = const.tile([P, P], bf16)
        make_identity(nc, ident[:])

        xgs = []
        for g in range(n_gtiles):
            xg = gpool.tile([P, DIM], bf16, name="xg", tag="xg", bufs=n_gtiles)
            gi_inst = nc.gpsimd.indirect_dma_start(
                out=xg[:],
                out_offset=None,
                in_=embed[:],
                in_offset=bass.IndirectOffsetOnAxis(ap=tok_sb[:, g : g + 1], axis=0),
            )
            if g % 2 == 1:
                gi_inst.ins.queue = "qPoolDynamic1"
            xgs.append(xg)

        # ---- constants / weights ----
        wih_sb = const.tile([P, J, 9, P], bf16)
        wih_s_sb = const.tile([P, J, 9, P], bf16)
        whh_sb = const.tile([P, J, 9, P], bf16)
        bias_sb = const.tile([P, 12], f32)
        bias3_sb = const.tile([3, 4, P], bf16)
        onehot3_sb = const.tile([3, 3, 512], bf16)
        nc.sync.dma_start(wih_sb[:], wih_t[:])
        nc.sync.dma_start(wih_s_sb[:], wih_s[:])
        nc.sync.dma_start(whh_sb[:], whh_t[:])
        nc.sync.dma_start(bias_sb[:], biases[:])
        nc.sync.dma_start(bias3_sb[:], biases_mm[:])
        nc.sync.dma_start(onehot3_sb[:], onehot3[:])

        x_in = {}
        for Pl in levels:
            x_in[Pl] = xpool.tile([P, J, ARITY, Pl], bf16, name=f"x{Pl}", tag=f"x{Pl}")

        # ---- transpose of gathered leaves happens inside the leaf level loop ----
        x5 = x_in[P5]

        def emit_transposes(g0, g1):
            for g in range(g0, g1):
                for j in range(J):
                    tp = pspool.tile([P, 512], bf16, name="tp", tag="ps")
                    nc.tensor.transpose(
                        tp[:, :P], xgs[g][:, j * P : (j + 1) * P], ident[:]
                    )
                    nc.vector.tensor_copy(
                        out=x5[:, j, :, 16 * g : 16 * (g + 1)],
                        in_=tp[:, :P].rearrange("p (par c) -> p c par", c=ARITY),
                    )

        def new_state(name, dtype, Pl):
            return state.tile([P, J, Pl], dtype, name=name, tag=name)

        def new_hacc(Pl):
            # permuted accumulator [128, J, 8, Pl/8] (child-major) when Pl >= 8
            if Pl >= ARITY:
                return state.tile([P, J, ARITY, Pl // ARITY], f32, name="hacc", tag="hacc")
            return state.tile([P, J, Pl], f32, name="hacc", tag="hacc")

        h = new_state("h", bf16, P5)
        hacc = new_hacc(P5)
        nc.gpsimd.memset(h[:], 0.0)
        nc.gpsimd.memset(hacc[:], 0.0)

        def psum_tile():
            return pspool.tile([P, 512], f32, name="ps", tag="ps")

        level_csum = [None]

        for li, Pl in enumerate(levels):
            with nc.named_scope(f"level_{Pl}"):
                xl = x_in[Pl]
                is_leaf = li == 0
                bias_in_psum = not is_leaf
                NCH = min(Pl, 256)
                nch = Pl // NCH
                g_per_ch = n_gtiles // nch
                for t in range(ARITY):
                    c = ARITY - 1 - t
                    skip_hh = is_leaf and t == 0
                    for ch in range(nch):
                        if is_leaf and t == 0:
                            emit_transposes(ch * g_per_ch, (ch + 1) * g_per_ch)
                        sl = slice(ch * NCH, (ch + 1) * NCH)
                        N3 = 3 * NCH

                        if bias_in_psum:
                            # one [128, 3*NCH] PSUM tile per role; the bias is a
                            # single K=3 matmul against a one-hot pattern that
                            # writes the whole tile (the only start=True), so
                            # gi/hh matmuls accumulate regardless of order.
                            ps_r, ps_z, ps_in = psum_tile(), psum_tile(), psum_tile()
                            ps_hn = None if skip_hh else psum_tile()

                            def view3(pst):
                                return pst[:, :N3].rearrange("p (j n) -> p j n", j=3)

                            def msl(pst, m):
                                return pst[:, m * NCH : (m + 1) * NCH]

                            role_list = [(ps_r, 0), (ps_z, 1), (ps_hn, 2), (ps_in, 3)]
                            for pst, ro in role_list:
                                if pst is None:
                                    continue
                                nc.tensor.matmul(
                                    pst[:, :N3],
                                    bias3_sb[:, ro, :],
                                    onehot3_sb[:, :, :NCH],
                                    start=True,
                                    stop=False,
                                )
                            # gi matmuls (no dependency on h)
                            for pst, moff in ((ps_r, 0), (ps_z, 3), (ps_in, 6)):
                                for m in range(3):
                                    for k in range(J):
                                        nc.tensor.matmul(
                                            msl(pst, m),
                                            wih_s_sb[:, k, moff + m, :],
                                            xl[:, k, c, sl],
                                            start=False,
                                            stop=(moff == 6 and m == 2 and k == 2),
                                        )
                            # hh matmuls
                            if not skip_hh:
                                for pst, moff in ((ps_r, 0), (ps_z, 3), (ps_hn, 6)):
                                    for m in range(3):
                                        for k in range(J):
                                            nc.tensor.matmul(
                                                msl(pst, m),
                                                whh_sb[:, k, moff + m, :],
                                                h[:, k, sl],
                                                start=False,
                                                stop=(m == 2 and k == 2),
                                            )
                        else:
                            # leaf level: per-m PSUM tiles (one bank each)
                            ps_r = [psum_tile()[:, :NCH] for _ in range(3)]
                            ps_z = [psum_tile()[:, :NCH] for _ in range(3)]
                            ps_in = [psum_tile()[:, :NCH] for _ in range(3)]
                            ps_hn = (
                                None
                                if skip_hh
                                else [psum_tile()[:, :NCH] for _ in range(3)]
                            )
                            for ps, moff in ((ps_r, 0), (ps_z, 3), (ps_in, 6)):
                                for m in range(3):
                                    for k in range(J):
                                        nc.tensor.matmul(
                                            ps[m],
                                            wih_sb[:, k, moff + m, :],
                                            xl[:, k, c, sl],
                                            start=(k == 0),
                                            stop=(k == 2 and (moff == 6 or skip_hh)),
                                        )
                            if not skip_hh:
                                for ps, moff in ((ps_r, 0), (ps_z, 3), (ps_hn, 6)):
                                    for m in range(3):
                                        for k in range(J):
                                            nc.tensor.matmul(
                                                ps[m],
                                                whh_sb[:, k, moff + m, :],
                                                h[:, k, sl],
                                                start=(k == 0 and moff == 6),
                                                stop=(k == 2),
                                            )

                        r_sb = gates.tile([P, J, NCH], bf16, name="r_sb", tag="r_sb")
                        z_sb = gates.tile([P, J, NCH], bf16, name="z_sb", tag="z_sb")
                        n_sb = gates.tile([P, J, NCH], bf16, name="n_sb", tag="n_sb")
                        rhn = gates.tile([P, J, NCH], f32, name="rhn", tag="rhn")
                        t1 = gates.tile([P, J, NCH], bf16, name="t1", tag="t1")

                        if bias_in_psum:
                            nc.scalar.activation(r_sb[:], view3(ps_r), Sig)
                            nc.scalar.activation(z_sb[:], view3(ps_z), Sig)
                            nc.vector.tensor_tensor(
                                out=rhn[:], in0=view3(ps_hn), in1=r_sb[:], op=Mult
                            )
                            nc.vector.tensor_tensor(
                                out=rhn[:], in0=rhn[:], in1=view3(ps_in), op=Add
                            )
                            nc.scalar.activation(n_sb[:], rhn[:], Tanh)
                        else:
                            for m in range(3):
                                nc.scalar.activation(
                                    r_sb[:, m], ps_r[m], Sig, bias=bias_sb[:, m : m + 1]
                                )
                            for m in range(3):
                                nc.scalar.activation(
                                    z_sb[:, m], ps_z[m], Sig, bias=bias_sb[:, 3 + m : 4 + m]
                                )
                            if skip_hh:
                                for m in range(3):
                                    nc.vector.tensor_scalar_mul(
                                        rhn[:, m], r_sb[:, m], bias_sb[:, 6 + m : 7 + m]
                                    )
                            else:
                                for m in range(3):
                                    nc.vector.scalar_tensor_tensor(
                                        out=rhn[:, m],
                                        in0=ps_hn[m],
                                        scalar=bias_sb[:, 6 + m : 7 + m],
                                        in1=r_sb[:, m],
                                        op0=Add,
                                        op1=Mult,
                                    )
                            for m in range(3):
                                nc.vector.tensor_tensor(
                                    out=rhn[:, m], in0=rhn[:, m], in1=ps_in[m], op=Add
                                )
                            for m in range(3):
                                nc.scalar.activation(
                                    n_sb[:, m], rhn[:, m], Tanh, bias=bias_sb[:, 9 + m : 10 + m]
                                )

                        # h = n + z * (h - n)
                        hsl = h[:, :, sl]
                        nc.vector.tensor_tensor(out=t1[:], in0=hsl, in1=n_sb[:], op=Sub)
                        nc.vector.tensor_tensor(out=t1[:], in0=z_sb[:], in1=t1[:], op=Mult)
                        nc.vector.tensor_tensor(out=hsl, in0=n_sb[:], in1=t1[:], op=Add)
                        if t == ARITY - 1 and Pl > 1:
                            hperm = hsl.rearrange("p j (q c) -> p j c q", c=ARITY)
                            qsl = slice(ch * NCH // ARITY, (ch + 1) * NCH // ARITY)
                            # child-mean of final hiddens for this chunk (h0 of
                            # the next level), before the x_next adds so the
                            # next level's recurrent matmuls unblock earlier
                            if ch == 0:
                                csum = state.tile(
                                    [P, J, Pl // ARITY], f32, name="csum", tag="csum"
                                )
                                level_csum[0] = csum
                            nc.vector.tensor_reduce(
                                out=level_csum[0][:, :, qsl],
                                in_=hsl.rearrange("p j (q c) -> p j q c", c=ARITY),
                                axis=mybir.AxisListType.X,
                                op=Add,
                            )
                            # final step: x_next = hacc + h (raw sum; the /8 is
                            # folded into wih_s), written straight into x_in
                            xn = x_in[Pl // ARITY]
                            for j in range(J):
                                eng = nc.gpsimd if j == 2 else nc.vector
                                eng.tensor_tensor(
                                    out=xn[:, j, :, qsl],
                                    in0=hacc[:, j, :, qsl],
                                    in1=hperm[:, j],
                                    op=Add,
                                )
                        elif Pl >= ARITY:
                            qsl = slice(ch * NCH // ARITY, (ch + 1) * NCH // ARITY)
                            nc.gpsimd.tensor_tensor(
                                out=hacc[:, :, :, qsl],
                                in0=hacc[:, :, :, qsl],
                                in1=hsl.rearrange("p j (q c) -> p j c q", c=ARITY),
                                op=Add,
                            )
                        else:
                            nc.gpsimd.tensor_tensor(
                                out=hacc[:, :, sl], in0=hacc[:, :, sl], in1=hsl, op=Add
                            )

                # ---- level epilogue ----
                if Pl > 1:
                    Pn = Pl // ARITY
                    csum = level_csum[0]
                    h = new_state("h", bf16, Pn)
                    hacc = new_hacc(Pn)
                    nc.scalar.mul(h[:], csum[:], 1.0 / ARITY)
                    nc.gpsimd.memset(hacc[:], 0.0)
                else:
                    xo = state.tile([P, J], f32, name="xo", tag="xo")
                    nc.scalar.mul(xo[:], hacc[:, :, 0], 1.0 / ARITY)
                    ho = state.tile([P, J], f32, name="ho", tag="ho")
                    nc.vector.tensor_copy(ho[:], h[:, :, 0])
                    nc.sync.dma_start(out_x[:], xo[:])
                    nc.sync.dma_start(out_h[:], ho[:])


def _build_program(n_leaves_core):
    if n_leaves_core in _PROG_CACHE:
        return _PROG_CACHE[n_leaves_core]
    import concourse.bacc as bacc
    import concourse.mybir as mybir
    import concourse.tile as tile

    f32 = mybir.dt.float32
    bf16 = mybir.dt.bfloat16

    nc = bacc.Bacc(
        "TRN2",
        target_bir_lowering=False,
        debug=False,
        enable_asserts=False,
        num_devices=NCORES,
        num_swdge_queues=2,
    )
    tokens = nc.dram_tensor("tokens", [n_leaves_core], mybir.dt.int32, kind="ExternalInput").ap()
    embed = nc.dram_tensor("embed", [VOCAB, DIM], bf16, kind="ExternalInput").ap()
    wih_t = nc.dram_tensor("wih_t", [P, J, 9, P], bf16, kind="ExternalInput").ap()
    wih_s = nc.dram_tensor("wih_s", [P, J, 9, P], bf16, kind="ExternalInput").ap()
    whh_t = nc.dram_tensor("whh_t", [P, J, 9, P], bf16, kind="ExternalInput").ap()
    biases = nc.dram_tensor("biases", [P, 12], f32, kind="ExternalInput").ap()
    biases_mm = nc.dram_tensor("biases_mm", [3, 4, P], bf16, kind="ExternalInput").ap()
    onehot3 = nc.dram_tensor("onehot3", [3, 3, 512], bf16, kind="ExternalInput").ap()
    out_x = nc.dram_tensor("out_x", [P, J], f32, kind="ExternalOutput").ap()
    out_h = nc.dram_tensor("out_h", [P, J], f32, kind="ExternalOutput").ap()

    with tile.TileContext(nc) as tc:
        _emit(
            tc,
            nc,
            (tokens, embed, wih_t, wih_s, whh_t, biases, biases_mm, onehot3, out_x, out_h),
            n_leaves_core,
        )
    nc.compile()
    _PROG_CACHE[n_leaves_core] = nc
    return nc


def _retile_weights(w):
    # w: [1152, 384] -> lhsT tiles [128(k_part), 3(k), 9(m), 128(m_col)] bf16
    wt = np.ascontiguousarray(w.T)  # [384, 1152]
    wt = wt.reshape(J, P, 9, P).transpose(1, 0, 2, 3)
    return np.ascontiguousarray(wt).astype(BF16)


def _prep_bias(b_ih, b_hh):
    biases = np.zeros((P, 12), np.float32)
    comb = (b_ih + b_hh).reshape(9, P)
    biases[:, 0:6] = comb[0:6].T
    biases[:, 6:9] = b_hh.reshape(9, P)[6:9].T
    biases[:, 9:12] = b_ih.reshape(9, P)[6:9].T
    return biases


def _prep_bias_mm(b_ih, b_hh):
    # lhsT[k, ro, q] = bias[q, 3*ro + k]: the K=3 bias matmul against the
    # one-hot rhs yields out[q, (j, n)] = bias[q, 3*ro + j].
    b = _prep_bias(b_ih, b_hh)  # [128, 12] cols: r0..2 z0..2 hn0..2 in0..2
    out = b.T.reshape(4, 3, P).transpose(1, 0, 2)
    return np.ascontiguousarray(out).astype(BF16)


def _prep_onehot3():
    out = np.zeros((3, 3, 512), np.float32)
    for k in range(3):
        out[k, k, :] = 1.0
    return out.astype(BF16)


def _gru_gates(x_t, h, w_ih, w_hh, b_ih, b_hh):
    gi = x_t @ w_ih.T + b_ih
    gh = h @ w_hh.T + b_hh
    i_r, i_z, i_n = np.split(gi, 3, axis=-1)
    h_r, h_z, h_n = np.split(gh, 3, axis=-1)
    r = 1.0 / (1.0 + np.exp(-(i_r + h_r)))
    z = 1.0 / (1.0 + np.exp(-(i_z + h_z)))
    n = np.tanh(i_n + r * h_n)
    return (1.0 - z) * n + z * h


def _root_gru(x_children, h0, w_ih, w_hh, b_ih, b_hh):
    h = h0.astype(np.float64)
    acc = np.zeros_like(h)
    for t in range(ARITY):
        x_t = x_children[ARITY - 1 - t].astype(np.float64)
        h = _gru_gates(x_t, h, w_ih.astype(np.float64), w_hh.astype(np.float64),
                       b_ih.astype(np.float64), b_hh.astype(np.float64))
        acc += h
    return (acc / ARITY).astype(np.float32)


def kernel(leaf_tokens, embed_table, w_ih, w_hh, b_ih, b_hh):
    from concourse.bass_utils import run_bass_kernel_spmd

    leaf_tokens = np.asarray(leaf_tokens, np.int32)
    embed_table = np.asarray(embed_table, np.float32)
    w_ih = np.asarray(w_ih, np.float32)
    w_hh = np.asarray(w_hh, np.float32)
    b_ih = np.asarray(b_ih, np.float32)
    b_hh = np.asarray(b_hh, np.float32)

    nc = _build_program(LEAVES_CORE)

    embed_bf = embed_table.astype(BF16)
    wih_t = _retile_weights(w_ih)
    wih_s = _retile_weights(w_ih / ARITY)
    whh_t = _retile_weights(w_hh)
    biases = _prep_bias(b_ih, b_hh)
    biases_mm = _prep_bias_mm(b_ih, b_hh)
    in_maps = []
    for core in range(NCORES):
        in_maps.append(
            {
                "tokens": np.ascontiguousarray(
                    leaf_tokens[core * LEAVES_CORE : (core + 1) * LEAVES_CORE]
                ),
                "embed": embed_bf,
                "wih_t": wih_t,
                "wih_s": wih_s,
                "whh_t": whh_t,
                "biases": biases,
                "biases_mm": biases_mm,
                "onehot3": _prep_onehot3(),
            }
        )
    res = run_bass_kernel_spmd(nc, in_maps, core_ids=list(range(NCORES)))

    xs = np.zeros((NCORES, DIM), np.float32)
    h8 = np.zeros((NCORES, DIM), np.float32)
    for core in range(NCORES):
        xs[core] = res.results[core]["out_x"].T.reshape(-1)
        h8[core] = res.results[core]["out_h"].T.reshape(-1)

    h0 = h8.mean(axis=0)
    out = _root_gru(xs, h0, w_ih, w_hh, b_ih, b_hh)
    return out.reshape(1, 1, DIM)
